# revision 26
# baseline (speedup 1.0000x reference)
"""Distributed Trainium2 kernel for the 3-branch masked attention problem.

Sharding: 8 cores; core c handles batch b = c//2 and heads h0 = 4*(c%2) .. +4.

Design (v2, Act-engine-bound):
- The exp volume (3 branches x 4 heads x 2048^2 / core) pins the Scalar (Act)
  engine at ~2.3us per (block, j) step; everything else is organized to hide
  under it.
- QKV: x host-pre-tiled [u, f, 128, 512], streamed once.  Per token-block u,
  single-PSUM-bank waves (KA, QA, VA, VPK) so attention PSUM pools can coexist;
  p/k q,k projections are deferred into the attention j-loop as injected work.
- Attention per (branch, I, j): 4 chained dots (one PE slot, full 128 rows),
  exp on Act (2 x [128,1024]), mask-multiply TT on DVE (fp8 mask), col-packed
  AV pairs (2 heads concurrent via col groups) + 4 packed rowsum matmuls.
- Epilogue: no PE transposes.  Rowsums -> reciprocal -> ones-select broadcast
  matmul -> TT normalize, accumulating into persistent [dv, tokens] fp32
  accumulators (otc2), already transposed for the output projection.
- Pairwise AllGather of each token half; T0 mid-attention (hidden), T1 at the
  tail; output projection of T0 injected into the j-loop.
"""

import numpy as np
import ml_dtypes

BF16 = ml_dtypes.bfloat16
FP8 = ml_dtypes.float8_e4m3fn

H = 8
DA, DP, DK = 2048, 1024, 1024
B, N = 4, 2048
DOUT = 512
H_LOC = 4
DA_H, DP_H, DK_H = DA // H, DP // H, DK // H      # 256, 128, 128
da, dp, dk = DA_H // H, DP_H // H, DK_H // H      # 32, 16, 16
DV = da + dp + dk                                 # 64
NCORES = 8

IB = 512            # query block
NJ = 16             # key chunks of 128
NI = N // IB        # 4

_CACHE = {}
import os
KCUT = int(os.environ.get("KCUT", "3"))  # 0=qkv 1=+attn 2=+epi 3=full


def _build():
    from collections import deque
    from types import SimpleNamespace

    import concourse.bass as bass  # noqa
    import concourse.mybir as mybir
    import concourse.tile as tile
    from concourse import bacc
    from concourse.masks import make_identity
    from concourse.tile import add_dep_helper

    f32 = mybir.dt.float32
    bf16 = mybir.dt.bfloat16
    fp8 = mybir.dt.float8e4
    Exp = mybir.ActivationFunctionType.Exp
    mult = mybir.AluOpType.mult
    add = mybir.AluOpType.add

    nc = bacc.Bacc("TRN2", target_bir_lowering=False, debug=False,
                   enable_asserts=False, num_devices=NCORES)

    # x pre-tiled on host: [u, f, 128, 512] -> [16384, 512]
    xT = nc.dram_tensor("xT", [4 * 32 * 128, 512], bf16, kind="ExternalInput")
    maskT = nc.dram_tensor("maskT", [N, N], fp8, kind="ExternalInput")
    wa = nc.dram_tensor("wa", [128, 16 * 384], bf16, kind="ExternalInput")
    wpk = nc.dram_tensor("wpk", [128, 16 * 384], bf16, kind="ExternalInput")
    wo = nc.dram_tensor("wo", [128, 4 * DOUT], bf16, kind="ExternalInput")
    bout = nc.dram_tensor("bout", [DOUT, 1], f32, kind="ExternalInput")
    selc = nc.dram_tensor("selc", [128, 256], bf16, kind="ExternalInput")
    out = nc.dram_tensor("out", [DOUT, N], bf16, kind="ExternalOutput")

    with tile.TileContext(nc) as tc:
        with (
            tc.tile_pool(name="const", bufs=1) as cpool,
            tc.tile_pool(name="dram", bufs=1, space="DRAM") as dpool,
            tc.tile_pool(name="xp", bufs=2) as xpool,
            tc.tile_pool(name="P1", bufs=2, space="PSUM") as P1,
            tc.tile_pool(name="P2", bufs=1, space="PSUM") as P2,
            tc.tile_pool(name="ep", bufs=2) as epool,
            tc.tile_pool(name="pp", bufs=5) as ppool,
            tc.tile_pool(name="of", bufs=1) as ofpool,
        ):
            # ---------------- constants / persistents ----------------
            ident_bf = cpool.tile([128, 128], bf16, name="ident_bf")
            make_identity(nc, ident_bf)

            bias_sb = cpool.tile([128, 4], f32, name="bias_sb")
            for t in range(4):
                nc.scalar.dma_start(bias_sb[:, t:t + 1],
                                    bout[128 * t:128 * (t + 1), :])

            wa_sb = cpool.tile([128, 16 * 384], bf16, name="wa_sb")
            nc.sync.dma_start(wa_sb[:], wa[:])
            wpk_sb = cpool.tile([128, 16 * 384], bf16, name="wpk_sb")
            nc.scalar.dma_start(wpk_sb[:], wpk[:])
            wo_sb = cpool.tile([128, 4 * DOUT], bf16, name="wo_sb")
            nc.scalar.dma_start(wo_sb[:], wo[:])

            ones1 = cpool.tile([128, 1], bf16, name="ones1")
            nc.gpsimd.memset(ones1[:], 1.0)
            # selN: row 32h holds ones at cols 64*(h%2)..+64 (host constant);
            # used as rank-1 stationary rows for the r-broadcast matmuls
            selN_sb = cpool.tile([128, 256], bf16, name="selN_sb")
            nc.scalar.dma_start(selN_sb[:], selc[:])
            # persistent r staging: rows other than 32h stay 1.0 forever so
            # the C=32 broadcast matmul contracts them against selN zeros
            rrP = cpool.tile([128, IB], f32, name="rrP")
            nc.gpsimd.memset(rrP[:], 1.0)
            rbP = cpool.tile([128, IB], f32, name="rbP")

            qTa = cpool.tile([128, N], bf16, name="qTa")
            kTa = cpool.tile([128, N], bf16, name="kTa")
            qTp = cpool.tile([128, N], bf16, name="qTp")
            kTp = cpool.tile([128, N], bf16, name="kTp")
            qTk = cpool.tile([128, N], bf16, name="qTk")
            kTk = cpool.tile([128, N], bf16, name="kTk")
            # V^T chunks transposed: vcomb[c][:, 128j : 128j+128] =
            #   [keys 128, (head 2c | head 2c+1) x 64]
            vcomb = [cpool.tile([128, N], bf16, name=f"vcomb{c}")
                     for c in range(2)]
            # normalized attention output accumulator, [2heads*64dv, tokens]
            otc2 = [cpool.tile([128, N], f32, name=f"otc2_{c}")
                    for c in range(2)]

            m_sb = [cpool.tile([128, N], fp8, name=f"m{j}") for j in range(NJ)]
            for j in range(NJ):
                nc.gpsimd.dma_start(m_sb[j][:],
                                    maskT[128 * j:128 * (j + 1), :])

            cc_in = [dpool.tile([2 * 128, N // 2], bf16, name=f"ccin{T}")
                     for T in range(2)]
            cc_out = [dpool.tile([4 * 128, N // 2], bf16, name=f"ccout{T}")
                      for T in range(2)]

            # ---------------- emission state ----------------
            state = dict(pend=[], p0=None, inject=deque())

            # ---------------- QKV waves ----------------
            def w_a(f, part):  # part: 0=q 1=k 2=v
                return wa_sb[:, 384 * f + 128 * part:384 * f + 128 * (part + 1)]

            def w_pk(i, part):  # i in 0..15: 0-7 = p chunks, 8-15 = k chunks
                base = 0 if i < 8 else 3072
                ii = i % 8
                return wpk_sb[:, base + 384 * ii + 128 * part:
                              base + 384 * ii + 128 * (part + 1)]

            def emit_wave(name, fs, wof, xts, dst):
                ps = P2.tile([128, IB], f32, tag="scratch", name=name)
                for k, f in enumerate(fs):
                    nc.tensor.matmul(ps[:], wof(f), xts[f][:],
                                     start=(k == 0), stop=(k == len(fs) - 1))
                dst(ps)

            def emit_u(u):
                """x DMAs + waves KA, QA, VA, VPK + vcomb transposes for u."""
                usl = slice(512 * u, 512 * (u + 1))
                xts = {}
                for f in range(32):
                    xt = xpool.tile([128, 512], bf16, tag=f"x{f}",
                                    bufs=(1 if f < 16 else 2),
                                    name=f"xt{f}")
                    nc.sync.dma_start(
                        xt[:], xT[(32 * u + f) * 128:(32 * u + f + 1) * 128, :])
                    xts[f] = xt

                emit_wave("ka", range(16), lambda f: w_a(f, 1), xts,
                          lambda ps: nc.vector.tensor_copy(kTa[:, usl], ps[:]))
                emit_wave("qa", range(16), lambda f: w_a(f, 0), xts,
                          lambda ps: nc.vector.tensor_copy(qTa[:, usl], ps[:]))

                comb = [ppool.tile([128, 512], bf16, tag=f"comb{c}", bufs=2,
                                   name=f"comb{c}") for c in range(2)]

                def va_dst(ps):
                    for h in range(H_LOC):
                        c, r0 = h // 2, 64 * (h % 2)
                        nc.vector.tensor_copy(comb[c][r0:r0 + 32, :],
                                              ps[32 * h:32 * h + 32, :])
                emit_wave("va", range(16), lambda f: w_a(f, 2), xts, va_dst)

                def vpk_dst(ps):
                    for h in range(H_LOC):
                        c, r0 = h // 2, 64 * (h % 2)
                        nc.vector.tensor_copy(comb[c][r0 + 32:r0 + 64, :],
                                              ps[32 * h:32 * h + 32, :])
                emit_wave("vpk", range(16, 32), lambda f: w_pk(f - 16, 2),
                          xts, vpk_dst)

                for jj in range(4):
                    j = 4 * u + jj
                    for c in range(2):
                        tp = P2.tile([128, 128], bf16, tag="scratch", name="tp")
                        nc.tensor.transpose(tp[:], comb[c][:, 128 * jj:128 * (jj + 1)],
                                            ident_bf[:])
                        nc.vector.tensor_copy(
                            vcomb[c][:, 128 * j:128 * (j + 1)], tp[:])
                return xts

            def enqueue_w2(u, xts):
                """p/k q,k projection waves as injected closures."""
                usl = slice(512 * u, 512 * (u + 1))
                specs = [("qp", range(16, 24), 0, qTp),
                         ("kp", range(16, 24), 1, kTp),
                         ("qk", range(24, 32), 0, qTk),
                         ("kk", range(24, 32), 1, kTk)]
                for name, fs, part, dstT in specs:
                    def go(name=name, fs=fs, part=part, dstT=dstT):
                        emit_wave(
                            name, fs, lambda f: w_pk(f - 16, part), xts,
                            lambda ps: nc.vector.tensor_copy(dstT[:, usl], ps[:]))
                    state["inject"].append(go)

            # ---------------- attention ----------------
            battn = [(qTa, kTa, da), (qTp, kTp, dp), (qTk, kTk, dk)]

            def mk_blk(bi, I):
                return SimpleNamespace(bi=bi, I=I,
                                       isl=slice(IB * I, IB * (I + 1)),
                                       qT=battn[bi][0], kT=battn[bi][1],
                                       d=battn[bi][2], oY=None, rP=None)

            def flush_pend():
                for blk, j, p0, p1 in state["pend"]:
                    if blk.oY is None:
                        blk.oY = [P2.tile([128, IB], f32, tag="oy", bufs=2,
                                          name=f"oY{c}") for c in range(2)]
                        blk.rP = P2.tile([128, IB], f32, tag="r", bufs=1,
                                         name="rP")
                    st, sp = (j == 0), (j == NJ - 1)
                    for half, p_sb in ((0, p0), (1, p1)):
                        prev = None
                        for hh in range(2):
                            mm = nc.tensor.matmul(
                                blk.oY[half][64 * hh:64 * (hh + 1), :],
                                vcomb[half][:, 128 * j + 64 * hh:
                                            128 * j + 64 * (hh + 1)],
                                p_sb[:, IB * hh:IB * (hh + 1)],
                                start=st, stop=sp, skip_group_check=True,
                                tile_position=(0, 64 * hh))
                            if prev is not None:
                                add_dep_helper(mm.ins, prev.ins, sync=False,
                                               reason="chain av")
                            prev = mm
                    prev = None
                    for h in range(H_LOC):
                        p_sb = p0 if h < 2 else p1
                        mm = nc.tensor.matmul(
                            blk.rP[32 * h:32 * h + 1, :],
                            ones1[:, 0:1],
                            p_sb[:, IB * (h % 2):IB * (h % 2 + 1)],
                            start=st, stop=sp, skip_group_check=True,
                            tile_position=(0, 32 * h))
                        if prev is not None:
                            add_dep_helper(mm.ins, prev.ins, sync=False,
                                           reason="chain rs")
                        prev = mm
                state["pend"] = []

            def make_epilogue(blk):
                def go():
                    for h in range(H_LOC):
                        nc.vector.tensor_copy(rrP[32 * h:32 * h + 1, :],
                                              blk.rP[32 * h:32 * h + 1, :])
                    nc.vector.reciprocal_approx_fast(rbP[:], rrP[:])
                    rbb = ppool.tile([128, IB], bf16, tag="rbb", bufs=1,
                                     name="rbb")
                    nc.vector.tensor_copy(rbb[:], rbP[:])
                    for c in range(2):
                        bc = P2.tile([128, IB], f32, tag="scratch", name="bc")
                        nc.tensor.matmul(
                            bc[:], selN_sb[:, 128 * c:128 * (c + 1)],
                            rbb[:], start=True, stop=True)
                        bcs = ppool.tile([128, IB], bf16, tag="bcs", bufs=1,
                                         name="bcs")
                        nc.vector.tensor_copy(bcs[:], bc[:])
                        if blk.bi == 0:
                            nc.vector.tensor_tensor(otc2[c][:, blk.isl],
                                                    blk.oY[c][:], bcs[:],
                                                    op=mult)
                        else:
                            tmp = ppool.tile([128, IB], f32, tag="tmp",
                                             bufs=1, name="tmp")
                            nc.vector.tensor_tensor(tmp[:], blk.oY[c][:],
                                                    bcs[:], op=mult)
                            nc.gpsimd.tensor_tensor(otc2[c][:, blk.isl],
                                                    otc2[c][:, blk.isl],
                                                    tmp[:], op=add)
                return go

            def stage_T(T):
                hsl = slice(1024 * T, 1024 * (T + 1))
                for c in range(2):
                    ob = ppool.tile([128, 1024], bf16, tag=f"ocb{c}", bufs=1,
                                    name=f"ocb{c}")
                    nc.vector.tensor_copy(ob[:], otc2[c][:, hsl])
                    nc.sync.dma_start(cc_in[T][128 * c:128 * (c + 1), :],
                                      ob[:])
                nc.gpsimd.collective_compute(
                    "AllGather", mybir.AluOpType.bypass,
                    replica_groups=[[0, 1], [2, 3], [4, 5], [6, 7]],
                    ins=[cc_in[T].opt()], outs=[cc_out[T].opt()])
                otf = []
                for c in range(4):
                    t = ofpool.tile([128, 1024], bf16, tag=f"otf{c}", bufs=1,
                                    name=f"otf{c}")
                    nc.sync.dma_start(t[:], cc_out[T][128 * c:128 * (c + 1), :])
                    otf.append(t)
                return otf

            def make_proj(T, otf, ot, i2):
                def go():
                    i2sl = slice(512 * i2, 512 * (i2 + 1))
                    ps = P2.tile([128, 512], f32, tag="scratch", name="pps")
                    for ic in range(4):
                        nc.tensor.matmul(
                            ps[:],
                            wo_sb[:, 512 * ic + 128 * ot:512 * ic + 128 * (ot + 1)],
                            otf[ic][:, i2sl], start=(ic == 0), stop=(ic == 3))
                    fin = epool.tile([128, 512], bf16, tag="fin", bufs=2,
                                     name="fin")
                    nc.vector.tensor_scalar_add(fin[:], ps[:],
                                                bias_sb[:, ot:ot + 1])
                    nc.sync.dma_start(
                        out[128 * ot:128 * (ot + 1),
                            1024 * T + 512 * i2:1024 * T + 512 * (i2 + 1)],
                        fin[:])
                return go

            def emit_j(blk, j, pop=0, pre=None):
                jsl = slice(128 * j, 128 * (j + 1))
                s_t = []
                prev = None
                for half in range(2):
                    s_ps = P1.tile([128, 2 * IB], f32, tag="s",
                                   name=f"sps{half}")
                    for hh in range(2):
                        h = 2 * half + hh
                        pb = 32 * h
                        mm = nc.tensor.matmul(
                            s_ps[:, IB * hh:IB * (hh + 1)],
                            blk.kT[pb:pb + blk.d, jsl],
                            blk.qT[pb:pb + blk.d, blk.isl],
                            start=True, stop=True, tile_position=(pb, 0))
                        if prev is not None:
                            add_dep_helper(mm.ins, prev.ins, sync=False,
                                           reason="chain dots")
                        prev = mm
                    s_t.append(s_ps)
                flush_pend()
                if pre is not None:
                    pre()
                for half in range(2):
                    e_sb = epool.tile([128, 2 * IB], bf16, tag="e", name="e_sb")
                    nc.scalar.activation(e_sb[:], s_t[half][:], Exp)
                    p_sb = ppool.tile([128, 2 * IB], bf16, tag="p", name="p_sb")
                    m_bc = m_sb[j][:, None, blk.isl].broadcast_to([128, 2, IB])
                    nc.vector.tensor_tensor(
                        p_sb[:].rearrange("p (g i) -> p g i", g=2),
                        e_sb[:].rearrange("p (g i) -> p g i", g=2),
                        m_bc, op=mult)
                    if half == 0:
                        state["p0"] = p_sb
                    else:
                        state["pend"].append((blk, j, state["p0"], p_sb))
                for _ in range(pop):
                    if state["inject"]:
                        state["inject"].popleft()()

            # ---------------- main flow ----------------
            blocks = [(bi, I) for bi in range(3) for I in range(NI)]
            if KCUT == 0:
                blocks = []
                for u in range(4):
                    xts = emit_u(u)
                    enqueue_w2(u, xts)
                while state["inject"]:
                    state["inject"].popleft()()
            pending_epi = None
            pending_extra = None

            for n, (bi, I) in enumerate(blocks):
                blk = mk_blk(bi, I)
                if n == 0:
                    xts0 = emit_u(0)
                    enqueue_w2(0, xts0)
                for j in range(NJ):
                    if n == 0 and j in (4, 8, 12):
                        xts = emit_u(j // 4)
                        enqueue_w2(j // 4, xts)
                    pre = None
                    if j == 0 and (pending_epi or pending_extra):
                        pe, px = pending_epi, pending_extra

                        def pre(pe=pe, px=px):
                            if pe:
                                pe()
                            if px:
                                px()
                        pending_epi = pending_extra = None
                    pop = 1 if ((n == 0 and j >= 4) or
                                (n >= 1 and j % 2 == 1)) else 0
                    emit_j(blk, j, pop=pop, pre=pre)
                if KCUT >= 2:
                    pending_epi = make_epilogue(blk)
                if KCUT >= 3 and n == 9:  # (2,1): token half T0 now final
                    def extra():
                        otf0 = stage_T(0)
                        for ot in range(4):
                            for i2 in range(2):
                                state["inject"].append(
                                    make_proj(0, otf0, ot, i2))
                    pending_extra = extra

            if blocks:
                flush_pend()
                if pending_epi:
                    pending_epi()
                while state["inject"]:
                    state["inject"].popleft()()
            if KCUT >= 3:
                otf1 = stage_T(1)
                for ot in range(4):
                    for i2 in range(2):
                        make_proj(1, otf1, ot, i2)()

    nc.compile()
    return nc


def _prep_core(c, x, W_a, W_p, W_k, W_out, b_out, mask):
    b = c // 2
    h0 = H_LOC * (c % 2)

    xTb = np.ascontiguousarray(x[b].T).astype(BF16)      # [4096, 2048]
    # tile into [u, f, 128, 512] -> [16384, 512]
    xt = xTb.reshape(32, 128, 4, 512).transpose(2, 0, 1, 3)
    xt = np.ascontiguousarray(xt.reshape(4 * 32 * 128, 512))

    maskTb = np.ascontiguousarray(mask[b, 0].T).astype(FP8)

    qa = W_a[da * h0: da * (h0 + H_LOC), :] * (DA ** -0.5)
    ka = W_a[DA_H + da * h0: DA_H + da * (h0 + H_LOC), :]
    va = W_a[2 * DA_H + da * h0: 2 * DA_H + da * (h0 + H_LOC), :]
    waT = np.concatenate([qa.T, ka.T, va.T], axis=1).astype(np.float32)
    # [2048, 384] -> stacked [128, 16*384]
    wa_all = waT.reshape(16, 128, 384).transpose(1, 0, 2).reshape(128, 6144)
    wa_all = np.ascontiguousarray(wa_all).astype(BF16)

    def pk_branch(W, D, D_H, d, vcol_ofs):
        qpad = np.zeros((D, 128), np.float32)
        kpad = np.zeros((D, 128), np.float32)
        vpad = np.zeros((D, 128), np.float32)
        for h in range(H_LOC):
            qpad[:, 32 * h:32 * h + d] = \
                W[d * (h0 + h): d * (h0 + h + 1), :].T * (D ** -0.5)
            kpad[:, 32 * h:32 * h + d] = \
                W[D_H + d * (h0 + h): D_H + d * (h0 + h + 1), :].T
            vpad[:, 32 * h + vcol_ofs:32 * h + vcol_ofs + d] = \
                W[2 * D_H + d * (h0 + h): 2 * D_H + d * (h0 + h + 1), :].T
        wT = np.concatenate([qpad, kpad, vpad], axis=1)   # [D, 384]
        return wT.reshape(8, 128, 384).transpose(1, 0, 2).reshape(128, 3072)

    wp_all = pk_branch(W_p, DP, DP_H, dp, 0)
    wk_all = pk_branch(W_k, DK, DK_H, dk, 16)
    wpk_all = np.ascontiguousarray(
        np.concatenate([wp_all, wk_all], axis=1)).astype(BF16)

    woutT = np.ascontiguousarray((W_out / 3.0).T).astype(np.float32)
    # [512, 512] -> [128, 4*512]
    wo_all = woutT.reshape(4, 128, 512).transpose(1, 0, 2).reshape(128, 2048)
    wo_all = np.ascontiguousarray(wo_all).astype(BF16)

    bout = np.ascontiguousarray(b_out.reshape(DOUT, 1)).astype(np.float32)

    # selc[:, 128c:128(c+1)] broadcasts r rows {32*2c, 32*(2c+1)} to the
    # 64-col blocks of bc: bc_c[m, q] = r[32*(2c + m//64)][q]
    selc = np.zeros((128, 256), np.float32)
    for c in range(2):
        for hh in range(2):
            selc[32 * (2 * c + hh), 128 * c + 64 * hh:
                 128 * c + 64 * hh + 64] = 1.0
    selc = np.ascontiguousarray(selc).astype(BF16)

    return {
        "selc": selc,
        "xT": xt,
        "maskT": maskTb,
        "wa": wa_all,
        "wpk": wpk_all,
        "wo": wo_all,
        "bout": bout,
    }


def kernel(x, W_a, W_p, W_k, W_out, b_out, mask):
    from concourse.bass_utils import run_bass_kernel_spmd

    x = np.asarray(x, np.float32)
    W_a = np.asarray(W_a, np.float32)
    W_p = np.asarray(W_p, np.float32)
    W_k = np.asarray(W_k, np.float32)
    W_out = np.asarray(W_out, np.float32)
    b_out = np.asarray(b_out, np.float32)
    mask = np.asarray(mask)

    if "nc" not in _CACHE:
        _CACHE["nc"] = _build()
    nc = _CACHE["nc"]

    in_maps = [_prep_core(c, x, W_a, W_p, W_k, W_out, b_out, mask)
               for c in range(NCORES)]
    res = run_bass_kernel_spmd(nc, in_maps, core_ids=list(range(NCORES)))

    outs = []
    for b in range(B):
        outs.append(np.asarray(res.results[2 * b]["out"]).astype(np.float32).T)
    return np.stack(outs, axis=0)


# revision 27
# speedup vs baseline: 1.3459x; 1.3459x over previous
"""Distributed Trainium2 kernel for the 3-branch masked attention problem.

Sharding: 8 cores; core c handles batch b = c//2 and heads h0 = 4*(c%2) .. +4.

Design (v2, Act-engine-bound):
- The exp volume (3 branches x 4 heads x 2048^2 / core) pins the Scalar (Act)
  engine at ~2.3us per (block, j) step; everything else is organized to hide
  under it.
- QKV: x host-pre-tiled [u, f, 128, 512], streamed once.  Per token-block u,
  single-PSUM-bank waves (KA, QA, VA, VPK) so attention PSUM pools can coexist;
  p/k q,k projections are deferred into the attention j-loop as injected work.
- Attention per (branch, I, j): 4 chained dots (one PE slot, full 128 rows),
  exp on Act (2 x [128,1024]), mask-multiply TT on DVE (fp8 mask), col-packed
  AV pairs (2 heads concurrent via col groups) + 4 packed rowsum matmuls.
- Epilogue: no PE transposes.  Rowsums -> reciprocal -> ones-select broadcast
  matmul -> TT normalize, accumulating into persistent [dv, tokens] fp32
  accumulators (otc2), already transposed for the output projection.
- Pairwise AllGather of each token half; T0 mid-attention (hidden), T1 at the
  tail; output projection of T0 injected into the j-loop.
"""

import numpy as np
import ml_dtypes

BF16 = ml_dtypes.bfloat16
FP8 = ml_dtypes.float8_e4m3fn

H = 8
DA, DP, DK = 2048, 1024, 1024
B, N = 4, 2048
DOUT = 512
H_LOC = 4
DA_H, DP_H, DK_H = DA // H, DP // H, DK // H      # 256, 128, 128
da, dp, dk = DA_H // H, DP_H // H, DK_H // H      # 32, 16, 16
DV = da + dp + dk                                 # 64
NCORES = 8

IB = 512            # query block
NJ = 16             # key chunks of 128
NI = N // IB        # 4

_CACHE = {}
import os
KCUT = int(os.environ.get("KCUT", "3"))  # 0=qkv 1=+attn 2=+epi 3=full


def _build():
    from collections import deque
    from types import SimpleNamespace

    import concourse.bass as bass  # noqa
    import concourse.mybir as mybir
    import concourse.tile as tile
    from concourse import bacc
    from concourse.masks import make_identity
    from concourse.tile import add_dep_helper

    f32 = mybir.dt.float32
    bf16 = mybir.dt.bfloat16
    fp8 = mybir.dt.float8e4
    Exp = mybir.ActivationFunctionType.Exp
    mult = mybir.AluOpType.mult
    add = mybir.AluOpType.add

    nc = bacc.Bacc("TRN2", target_bir_lowering=False, debug=False,
                   enable_asserts=False, num_devices=NCORES)

    # x pre-tiled on host: [u, f, 128, 512] -> [16384, 512]
    xT = nc.dram_tensor("xT", [4 * 32 * 128, 512], bf16, kind="ExternalInput")
    maskT = nc.dram_tensor("maskT", [N, N], bf16, kind="ExternalInput")
    wa = nc.dram_tensor("wa", [128, 16 * 384], bf16, kind="ExternalInput")
    wpk = nc.dram_tensor("wpk", [128, 16 * 384], bf16, kind="ExternalInput")
    wo = nc.dram_tensor("wo", [128, 4 * DOUT], bf16, kind="ExternalInput")
    bout = nc.dram_tensor("bout", [DOUT, 1], f32, kind="ExternalInput")
    selc = nc.dram_tensor("selc", [128, 256], bf16, kind="ExternalInput")
    out = nc.dram_tensor("out", [DOUT, N], bf16, kind="ExternalOutput")

    with tile.TileContext(nc) as tc:
        with (
            tc.tile_pool(name="const", bufs=1) as cpool,
            tc.tile_pool(name="dram", bufs=1, space="DRAM") as dpool,
            tc.tile_pool(name="xp", bufs=2) as xpool,
            tc.tile_pool(name="P1", bufs=2, space="PSUM") as P1,
            tc.tile_pool(name="P2", bufs=1, space="PSUM") as P2,
            tc.tile_pool(name="ep", bufs=2) as epool,
            tc.tile_pool(name="pp", bufs=5) as ppool,
            tc.tile_pool(name="of", bufs=1) as ofpool,
        ):
            # ---------------- constants / persistents ----------------
            ident_bf = cpool.tile([128, 128], bf16, name="ident_bf")
            make_identity(nc, ident_bf)

            bias_sb = cpool.tile([128, 4], f32, name="bias_sb")
            for t in range(4):
                nc.scalar.dma_start(bias_sb[:, t:t + 1],
                                    bout[128 * t:128 * (t + 1), :])

            wa_sb = cpool.tile([128, 16 * 384], bf16, name="wa_sb")
            nc.sync.dma_start(wa_sb[:], wa[:])
            wpk_sb = cpool.tile([128, 16 * 384], bf16, name="wpk_sb")
            nc.scalar.dma_start(wpk_sb[:], wpk[:])
            wo_sb = cpool.tile([128, 4 * DOUT], bf16, name="wo_sb")
            nc.scalar.dma_start(wo_sb[:], wo[:])

            ones1 = cpool.tile([128, 1], bf16, name="ones1")
            nc.gpsimd.memset(ones1[:], 1.0)
            # selN: row 32h holds ones at cols 64*(h%2)..+64 (host constant);
            # used as rank-1 stationary rows for the r-broadcast matmuls
            selN_sb = cpool.tile([128, 256], bf16, name="selN_sb")
            nc.scalar.dma_start(selN_sb[:], selc[:])
            # persistent r staging: rows other than 32h stay 1.0 forever so
            # the C=32 broadcast matmul contracts them against selN zeros
            rrP = cpool.tile([128, IB], f32, name="rrP")
            nc.gpsimd.memset(rrP[:], 1.0)
            rbP = cpool.tile([128, IB], f32, name="rbP")

            qTa = cpool.tile([128, N], bf16, name="qTa")
            kTa = cpool.tile([128, N], bf16, name="kTa")
            qTp = cpool.tile([128, N], bf16, name="qTp")
            kTp = cpool.tile([128, N], bf16, name="kTp")
            qTk = cpool.tile([128, N], bf16, name="qTk")
            kTk = cpool.tile([128, N], bf16, name="kTk")
            # V^T chunks transposed: vcomb[c][:, 128j : 128j+128] =
            #   [keys 128, (head 2c | head 2c+1) x 64]
            vcomb = [cpool.tile([128, N], bf16, name=f"vcomb{c}")
                     for c in range(2)]
            # normalized attention output accumulator, [2heads*64dv, tokens]
            otc2 = [cpool.tile([128, N], f32, name=f"otc2_{c}")
                    for c in range(2)]


            cc_in = [dpool.tile([2 * 128, N // 2], bf16, name=f"ccin{T}")
                     for T in range(2)]
            cc_out = [dpool.tile([4 * 128, N // 2], bf16, name=f"ccout{T}")
                      for T in range(2)]

            # ---------------- emission state ----------------
            state = dict(pend=[], p0=None, inject=deque())

            # ---------------- QKV waves ----------------
            def w_a(f, part):  # part: 0=q 1=k 2=v
                return wa_sb[:, 384 * f + 128 * part:384 * f + 128 * (part + 1)]

            def w_pk(i, part):  # i in 0..15: 0-7 = p chunks, 8-15 = k chunks
                base = 0 if i < 8 else 3072
                ii = i % 8
                return wpk_sb[:, base + 384 * ii + 128 * part:
                              base + 384 * ii + 128 * (part + 1)]

            def emit_wave(name, fs, wof, xts, dst):
                ps = P2.tile([128, IB], f32, tag="scratch", name=name)
                for k, f in enumerate(fs):
                    nc.tensor.matmul(ps[:], wof(f), xts[f][:],
                                     start=(k == 0), stop=(k == len(fs) - 1))
                dst(ps)

            def emit_u(u):
                """x DMAs + waves KA, QA, VA, VPK + vcomb transposes for u."""
                usl = slice(512 * u, 512 * (u + 1))
                xts = {}
                for f in range(32):
                    xt = xpool.tile([128, 512], bf16, tag=f"x{f}", bufs=2,
                                    name=f"xt{f}")
                    nc.sync.dma_start(
                        xt[:], xT[(32 * u + f) * 128:(32 * u + f + 1) * 128, :])
                    xts[f] = xt

                emit_wave("ka", range(16), lambda f: w_a(f, 1), xts,
                          lambda ps: nc.vector.tensor_copy(kTa[:, usl], ps[:]))
                emit_wave("qa", range(16), lambda f: w_a(f, 0), xts,
                          lambda ps: nc.vector.tensor_copy(qTa[:, usl], ps[:]))

                comb = [ppool.tile([128, 512], bf16, tag=f"comb{c}", bufs=2,
                                   name=f"comb{c}") for c in range(2)]

                def va_dst(ps):
                    for h in range(H_LOC):
                        c, r0 = h // 2, 64 * (h % 2)
                        nc.vector.tensor_copy(comb[c][r0:r0 + 32, :],
                                              ps[32 * h:32 * h + 32, :])
                emit_wave("va", range(16), lambda f: w_a(f, 2), xts, va_dst)

                def vpk_dst(ps):
                    for h in range(H_LOC):
                        c, r0 = h // 2, 64 * (h % 2)
                        nc.vector.tensor_copy(comb[c][r0 + 32:r0 + 64, :],
                                              ps[32 * h:32 * h + 32, :])
                emit_wave("vpk", range(16, 32), lambda f: w_pk(f - 16, 2),
                          xts, vpk_dst)

                for jj in range(4):
                    j = 4 * u + jj
                    for c in range(2):
                        tp = P2.tile([128, 128], bf16, tag="scratch", name="tp")
                        nc.tensor.transpose(tp[:], comb[c][:, 128 * jj:128 * (jj + 1)],
                                            ident_bf[:])
                        nc.vector.tensor_copy(
                            vcomb[c][:, 128 * j:128 * (j + 1)], tp[:])
                return xts

            def enqueue_w2(u, xts):
                """p/k q,k projection waves as injected closures."""
                usl = slice(512 * u, 512 * (u + 1))
                specs = [("qp", range(16, 24), 0, qTp),
                         ("kp", range(16, 24), 1, kTp),
                         ("qk", range(24, 32), 0, qTk),
                         ("kk", range(24, 32), 1, kTk)]
                for name, fs, part, dstT in specs:
                    def go(name=name, fs=fs, part=part, dstT=dstT):
                        emit_wave(
                            name, fs, lambda f: w_pk(f - 16, part), xts,
                            lambda ps: nc.vector.tensor_copy(dstT[:, usl], ps[:]))
                    state["inject"].append(go)

            # ---------------- attention ----------------
            battn = [(qTa, kTa, da), (qTp, kTp, dp), (qTk, kTk, dk)]

            def mk_blk(bi, I):
                return SimpleNamespace(bi=bi, I=I,
                                       isl=slice(IB * I, IB * (I + 1)),
                                       qT=battn[bi][0], kT=battn[bi][1],
                                       d=battn[bi][2], oY=None, rP=None)

            def flush_pend():
                for blk, j, p0, p1 in state["pend"]:
                    if blk.oY is None:
                        blk.oY = [P2.tile([128, IB], f32, tag="oy", bufs=2,
                                          name=f"oY{c}") for c in range(2)]
                        blk.rP = P2.tile([128, IB], f32, tag="r", bufs=1,
                                         name="rP")
                    st, sp = (j == 0), (j == NJ - 1)
                    for half, p_sb in ((0, p0), (1, p1)):
                        prev = None
                        for hh in range(2):
                            mm = nc.tensor.matmul(
                                blk.oY[half][64 * hh:64 * (hh + 1), :],
                                vcomb[half][:, 128 * j + 64 * hh:
                                            128 * j + 64 * (hh + 1)],
                                p_sb[:, IB * hh:IB * (hh + 1)],
                                start=st, stop=sp, skip_group_check=True,
                                tile_position=(0, 64 * hh))
                            if prev is not None:
                                add_dep_helper(mm.ins, prev.ins, sync=False,
                                               reason="chain av")
                            prev = mm
                    prev = None
                    for h in range(H_LOC):
                        p_sb = p0 if h < 2 else p1
                        mm = nc.tensor.matmul(
                            blk.rP[32 * h:32 * h + 1, :],
                            ones1[:, 0:1],
                            p_sb[:, IB * (h % 2):IB * (h % 2 + 1)],
                            start=st, stop=sp, skip_group_check=True,
                            tile_position=(0, 32 * h))
                        if prev is not None:
                            add_dep_helper(mm.ins, prev.ins, sync=False,
                                           reason="chain rs")
                        prev = mm
                state["pend"] = []

            def make_epilogue(blk):
                def go():
                    for h in range(H_LOC):
                        nc.vector.tensor_copy(rrP[32 * h:32 * h + 1, :],
                                              blk.rP[32 * h:32 * h + 1, :])
                    nc.vector.reciprocal_approx_fast(rbP[:], rrP[:])
                    rbb = ppool.tile([128, IB], bf16, tag="rbb", bufs=1,
                                     name="rbb")
                    nc.vector.tensor_copy(rbb[:], rbP[:])
                    for c in range(2):
                        bc = P2.tile([128, IB], f32, tag="scratch", name="bc")
                        nc.tensor.matmul(
                            bc[:], selN_sb[:, 128 * c:128 * (c + 1)],
                            rbb[:], start=True, stop=True)
                        bcs = ppool.tile([128, IB], bf16, tag="bcs", bufs=1,
                                         name="bcs")
                        nc.vector.tensor_copy(bcs[:], bc[:])
                        if blk.bi == 0:
                            nc.vector.tensor_tensor(otc2[c][:, blk.isl],
                                                    blk.oY[c][:], bcs[:],
                                                    op=mult)
                        else:
                            tmp = ppool.tile([128, IB], f32, tag="tmp",
                                             bufs=1, name="tmp")
                            nc.vector.tensor_tensor(tmp[:], blk.oY[c][:],
                                                    bcs[:], op=mult)
                            nc.gpsimd.tensor_tensor(otc2[c][:, blk.isl],
                                                    otc2[c][:, blk.isl],
                                                    tmp[:], op=add)
                return go

            def stage_T(T):
                hsl = slice(1024 * T, 1024 * (T + 1))
                for c in range(2):
                    ob = ppool.tile([128, 1024], bf16, tag=f"ocb{c}", bufs=1,
                                    name=f"ocb{c}")
                    nc.vector.tensor_copy(ob[:], otc2[c][:, hsl])
                    nc.sync.dma_start(cc_in[T][128 * c:128 * (c + 1), :],
                                      ob[:])
                nc.gpsimd.collective_compute(
                    "AllGather", mybir.AluOpType.bypass,
                    replica_groups=[[0, 1], [2, 3], [4, 5], [6, 7]],
                    ins=[cc_in[T].opt()], outs=[cc_out[T].opt()])
                otf = []
                for c in range(4):
                    t = ofpool.tile([128, 1024], bf16, tag=f"otf{c}", bufs=1,
                                    name=f"otf{c}")
                    nc.sync.dma_start(t[:], cc_out[T][128 * c:128 * (c + 1), :])
                    otf.append(t)
                return otf

            def make_proj(T, otf, ot, i2):
                def go():
                    i2sl = slice(512 * i2, 512 * (i2 + 1))
                    ps = P2.tile([128, 512], f32, tag="scratch", name="pps")
                    for ic in range(4):
                        nc.tensor.matmul(
                            ps[:],
                            wo_sb[:, 512 * ic + 128 * ot:512 * ic + 128 * (ot + 1)],
                            otf[ic][:, i2sl], start=(ic == 0), stop=(ic == 3))
                    fin = epool.tile([128, 512], bf16, tag="fin", bufs=2,
                                     name="fin")
                    nc.vector.tensor_scalar_add(fin[:], ps[:],
                                                bias_sb[:, ot:ot + 1])
                    nc.sync.dma_start(
                        out[128 * ot:128 * (ot + 1),
                            1024 * T + 512 * i2:1024 * T + 512 * (i2 + 1)],
                        fin[:])
                return go

            def emit_j(blk, j, pop=0, pre=None):
                jsl = slice(128 * j, 128 * (j + 1))
                m_t = ppool.tile([128, IB], bf16, tag="m", bufs=4, name="m_t")
                nc.gpsimd.dma_start(m_t[:], maskT[jsl, blk.isl])
                s_t = []
                prev = None
                for half in range(2):
                    s_ps = P1.tile([128, 2 * IB], f32, tag="s",
                                   name=f"sps{half}")
                    for hh in range(2):
                        h = 2 * half + hh
                        pb = 32 * h
                        mm = nc.tensor.matmul(
                            s_ps[:, IB * hh:IB * (hh + 1)],
                            blk.kT[pb:pb + blk.d, jsl],
                            blk.qT[pb:pb + blk.d, blk.isl],
                            start=True, stop=True, tile_position=(pb, 0))
                        if prev is not None:
                            add_dep_helper(mm.ins, prev.ins, sync=False,
                                           reason="chain dots")
                        prev = mm
                    s_t.append(s_ps)
                flush_pend()
                if pre is not None:
                    pre()
                for half in range(2):
                    e_sb = epool.tile([128, 2 * IB], bf16, tag="e", name="e_sb")
                    nc.scalar.activation(e_sb[:], s_t[half][:], Exp)
                    p_sb = ppool.tile([128, 2 * IB], bf16, tag="p", name="p_sb")
                    m_bc = m_t[:, None, :].broadcast_to([128, 2, IB])
                    nc.vector.tensor_tensor(
                        p_sb[:].rearrange("p (g i) -> p g i", g=2),
                        e_sb[:].rearrange("p (g i) -> p g i", g=2),
                        m_bc, op=mult)
                    if half == 0:
                        state["p0"] = p_sb
                    else:
                        state["pend"].append((blk, j, state["p0"], p_sb))
                for _ in range(pop):
                    if state["inject"]:
                        state["inject"].popleft()()

            # ---------------- main flow ----------------
            blocks = [(bi, I) for bi in range(3) for I in range(NI)]
            if KCUT == 0:
                blocks = []
                for u in range(4):
                    xts = emit_u(u)
                    enqueue_w2(u, xts)
                while state["inject"]:
                    state["inject"].popleft()()
            pending_epi = None
            pending_extra = None

            for n, (bi, I) in enumerate(blocks):
                blk = mk_blk(bi, I)
                if n == 0:
                    xts0 = emit_u(0)
                    enqueue_w2(0, xts0)
                for j in range(NJ):
                    if n == 0 and j in (4, 8, 12):
                        xts = emit_u(j // 4)
                        enqueue_w2(j // 4, xts)
                    pre = None
                    if j == 0 and (pending_epi or pending_extra):
                        pe, px = pending_epi, pending_extra

                        def pre(pe=pe, px=px):
                            if pe:
                                pe()
                            if px:
                                px()
                        pending_epi = pending_extra = None
                    pop = 1 if ((n == 0 and j >= 4) or
                                (n >= 1 and j % 2 == 1)) else 0
                    emit_j(blk, j, pop=pop, pre=pre)
                if KCUT >= 2:
                    pending_epi = make_epilogue(blk)
                if KCUT >= 3 and n == 9:  # (2,1): token half T0 now final
                    def extra():
                        otf0 = stage_T(0)
                        for ot in range(4):
                            for i2 in range(2):
                                state["inject"].append(
                                    make_proj(0, otf0, ot, i2))
                    pending_extra = extra

            if blocks:
                flush_pend()
                if pending_epi:
                    pending_epi()
                while state["inject"]:
                    state["inject"].popleft()()
            if KCUT >= 3:
                otf1 = stage_T(1)
                for ot in range(4):
                    for i2 in range(2):
                        make_proj(1, otf1, ot, i2)()

    nc.compile()
    return nc


def _prep_core(c, x, W_a, W_p, W_k, W_out, b_out, mask):
    b = c // 2
    h0 = H_LOC * (c % 2)

    xTb = np.ascontiguousarray(x[b].T).astype(BF16)      # [4096, 2048]
    # tile into [u, f, 128, 512] -> [16384, 512]
    xt = xTb.reshape(32, 128, 4, 512).transpose(2, 0, 1, 3)
    xt = np.ascontiguousarray(xt.reshape(4 * 32 * 128, 512))

    maskTb = np.ascontiguousarray(mask[b, 0].T).astype(BF16)

    qa = W_a[da * h0: da * (h0 + H_LOC), :] * (DA ** -0.5)
    ka = W_a[DA_H + da * h0: DA_H + da * (h0 + H_LOC), :]
    va = W_a[2 * DA_H + da * h0: 2 * DA_H + da * (h0 + H_LOC), :]
    waT = np.concatenate([qa.T, ka.T, va.T], axis=1).astype(np.float32)
    # [2048, 384] -> stacked [128, 16*384]
    wa_all = waT.reshape(16, 128, 384).transpose(1, 0, 2).reshape(128, 6144)
    wa_all = np.ascontiguousarray(wa_all).astype(BF16)

    def pk_branch(W, D, D_H, d, vcol_ofs):
        qpad = np.zeros((D, 128), np.float32)
        kpad = np.zeros((D, 128), np.float32)
        vpad = np.zeros((D, 128), np.float32)
        for h in range(H_LOC):
            qpad[:, 32 * h:32 * h + d] = \
                W[d * (h0 + h): d * (h0 + h + 1), :].T * (D ** -0.5)
            kpad[:, 32 * h:32 * h + d] = \
                W[D_H + d * (h0 + h): D_H + d * (h0 + h + 1), :].T
            vpad[:, 32 * h + vcol_ofs:32 * h + vcol_ofs + d] = \
                W[2 * D_H + d * (h0 + h): 2 * D_H + d * (h0 + h + 1), :].T
        wT = np.concatenate([qpad, kpad, vpad], axis=1)   # [D, 384]
        return wT.reshape(8, 128, 384).transpose(1, 0, 2).reshape(128, 3072)

    wp_all = pk_branch(W_p, DP, DP_H, dp, 0)
    wk_all = pk_branch(W_k, DK, DK_H, dk, 16)
    wpk_all = np.ascontiguousarray(
        np.concatenate([wp_all, wk_all], axis=1)).astype(BF16)

    woutT = np.ascontiguousarray((W_out / 3.0).T).astype(np.float32)
    # [512, 512] -> [128, 4*512]
    wo_all = woutT.reshape(4, 128, 512).transpose(1, 0, 2).reshape(128, 2048)
    wo_all = np.ascontiguousarray(wo_all).astype(BF16)

    bout = np.ascontiguousarray(b_out.reshape(DOUT, 1)).astype(np.float32)

    # selc[:, 128c:128(c+1)] broadcasts r rows {32*2c, 32*(2c+1)} to the
    # 64-col blocks of bc: bc_c[m, q] = r[32*(2c + m//64)][q]
    selc = np.zeros((128, 256), np.float32)
    for c in range(2):
        for hh in range(2):
            selc[32 * (2 * c + hh), 128 * c + 64 * hh:
                 128 * c + 64 * hh + 64] = 1.0
    selc = np.ascontiguousarray(selc).astype(BF16)

    return {
        "selc": selc,
        "xT": xt,
        "maskT": maskTb,
        "wa": wa_all,
        "wpk": wpk_all,
        "wo": wo_all,
        "bout": bout,
    }


def kernel(x, W_a, W_p, W_k, W_out, b_out, mask):
    from concourse.bass_utils import run_bass_kernel_spmd

    x = np.asarray(x, np.float32)
    W_a = np.asarray(W_a, np.float32)
    W_p = np.asarray(W_p, np.float32)
    W_k = np.asarray(W_k, np.float32)
    W_out = np.asarray(W_out, np.float32)
    b_out = np.asarray(b_out, np.float32)
    mask = np.asarray(mask)

    if "nc" not in _CACHE:
        _CACHE["nc"] = _build()
    nc = _CACHE["nc"]

    in_maps = [_prep_core(c, x, W_a, W_p, W_k, W_out, b_out, mask)
               for c in range(NCORES)]
    res = run_bass_kernel_spmd(nc, in_maps, core_ids=list(range(NCORES)))

    outs = []
    for b in range(B):
        outs.append(np.asarray(res.results[2 * b]["out"]).astype(np.float32).T)
    return np.stack(outs, axis=0)


# revision 29
# speedup vs baseline: 1.4045x; 1.0436x over previous
"""Distributed Trainium2 kernel for the 3-branch masked attention problem.

Sharding: 8 cores; core c handles batch b = c//2 and heads h0 = 4*(c%2) .. +4.

Design (v2, Act-engine-bound):
- The exp volume (3 branches x 4 heads x 2048^2 / core) pins the Scalar (Act)
  engine at ~2.3us per (block, j) step; everything else is organized to hide
  under it.
- QKV: x host-pre-tiled [u, f, 128, 512], streamed once.  Per token-block u,
  single-PSUM-bank waves (KA, QA, VA, VPK) so attention PSUM pools can coexist;
  p/k q,k projections are deferred into the attention j-loop as injected work.
- Attention per (branch, I, j): 4 chained dots (one PE slot, full 128 rows),
  exp on Act (2 x [128,1024]), mask-multiply TT on DVE (fp8 mask), col-packed
  AV pairs (2 heads concurrent via col groups) + 4 packed rowsum matmuls.
- Epilogue: no PE transposes.  Rowsums -> reciprocal -> ones-select broadcast
  matmul -> TT normalize, accumulating into persistent [dv, tokens] fp32
  accumulators (otc2), already transposed for the output projection.
- Pairwise AllGather of each token half; T0 mid-attention (hidden), T1 at the
  tail; output projection of T0 injected into the j-loop.
"""

import numpy as np
import ml_dtypes

BF16 = ml_dtypes.bfloat16
FP8 = ml_dtypes.float8_e4m3fn

H = 8
DA, DP, DK = 2048, 1024, 1024
B, N = 4, 2048
DOUT = 512
H_LOC = 4
DA_H, DP_H, DK_H = DA // H, DP // H, DK // H      # 256, 128, 128
da, dp, dk = DA_H // H, DP_H // H, DK_H // H      # 32, 16, 16
DV = da + dp + dk                                 # 64
NCORES = 8

IB = 512            # query block
NJ = 16             # key chunks of 128
NI = N // IB        # 4

_CACHE = {}
import os
KCUT = int(os.environ.get("KCUT", "3"))  # 0=qkv 1=+attn 2=+epi 3=full


def _build():
    from collections import deque
    from types import SimpleNamespace

    import concourse.bass as bass  # noqa
    import concourse.mybir as mybir
    import concourse.tile as tile
    from concourse import bacc
    from concourse.masks import make_identity
    from concourse.tile import add_dep_helper

    f32 = mybir.dt.float32
    bf16 = mybir.dt.bfloat16
    fp8 = mybir.dt.float8e4
    Exp = mybir.ActivationFunctionType.Exp
    mult = mybir.AluOpType.mult
    add = mybir.AluOpType.add

    nc = bacc.Bacc("TRN2", target_bir_lowering=False, debug=False,
                   enable_asserts=False, num_devices=NCORES)

    # x pre-tiled on host: [u, f, 128, 512] -> [16384, 512]
    xT = nc.dram_tensor("xT", [4 * 32 * 128, 512], bf16, kind="ExternalInput")
    maskT = nc.dram_tensor("maskT", [N, N], bf16, kind="ExternalInput")
    wa = nc.dram_tensor("wa", [128, 16 * 384], bf16, kind="ExternalInput")
    wpk = nc.dram_tensor("wpk", [128, 16 * 384], bf16, kind="ExternalInput")
    wo = nc.dram_tensor("wo", [128, 4 * DOUT], bf16, kind="ExternalInput")
    bout = nc.dram_tensor("bout", [DOUT, 1], f32, kind="ExternalInput")
    selc = nc.dram_tensor("selc", [128, 256], bf16, kind="ExternalInput")
    out = nc.dram_tensor("out", [DOUT, N], bf16, kind="ExternalOutput")

    with tile.TileContext(nc) as tc:
        with (
            tc.tile_pool(name="const", bufs=1) as cpool,
            tc.tile_pool(name="dram", bufs=1, space="DRAM") as dpool,
            tc.tile_pool(name="xp", bufs=2) as xpool,
            tc.tile_pool(name="P1", bufs=2, space="PSUM") as P1,
            tc.tile_pool(name="P2", bufs=1, space="PSUM") as P2,
            tc.tile_pool(name="ep", bufs=2) as epool,
            tc.tile_pool(name="pp", bufs=5) as ppool,
            tc.tile_pool(name="of", bufs=1) as ofpool,
        ):
            # ---------------- constants / persistents ----------------
            ident_bf = cpool.tile([128, 128], bf16, name="ident_bf")
            make_identity(nc, ident_bf)

            bias_sb = cpool.tile([128, 4], f32, name="bias_sb")
            for t in range(4):
                nc.scalar.dma_start(bias_sb[:, t:t + 1],
                                    bout[128 * t:128 * (t + 1), :])

            wa_sb = cpool.tile([128, 16 * 384], bf16, name="wa_sb")
            nc.sync.dma_start(wa_sb[:], wa[:])
            wpk_sb = cpool.tile([128, 16 * 384], bf16, name="wpk_sb")
            nc.scalar.dma_start(wpk_sb[:], wpk[:])
            wo_sb = cpool.tile([128, 4 * DOUT], bf16, name="wo_sb")
            nc.scalar.dma_start(wo_sb[:], wo[:])

            ones1 = cpool.tile([128, 1], bf16, name="ones1")
            nc.gpsimd.memset(ones1[:], 1.0)
            # selN: row 32h holds ones at cols 64*(h%2)..+64 (host constant);
            # used as rank-1 stationary rows for the r-broadcast matmuls
            selN_sb = cpool.tile([128, 256], bf16, name="selN_sb")
            nc.scalar.dma_start(selN_sb[:], selc[:])
            # persistent r staging: rows other than 32h stay 1.0 forever so
            # the C=32 broadcast matmul contracts them against selN zeros
            rrP = cpool.tile([128, IB], f32, name="rrP")
            nc.gpsimd.memset(rrP[:], 1.0)
            rbP = cpool.tile([128, IB], f32, name="rbP")

            qTa = cpool.tile([128, N], bf16, name="qTa")
            kTa = cpool.tile([128, N], bf16, name="kTa")
            qTp = cpool.tile([128, N], bf16, name="qTp")
            kTp = cpool.tile([128, N], bf16, name="kTp")
            qTk = cpool.tile([128, N], bf16, name="qTk")
            kTk = cpool.tile([128, N], bf16, name="kTk")
            # V^T chunks transposed: vcomb[c][:, 128j : 128j+128] =
            #   [keys 128, (head 2c | head 2c+1) x 64]
            vcomb = [cpool.tile([128, N], bf16, name=f"vcomb{c}")
                     for c in range(2)]
            # normalized attention output accumulator, [2heads*64dv, tokens]
            otc2 = [cpool.tile([128, N], f32, name=f"otc2_{c}")
                    for c in range(2)]


            cc_in = [dpool.tile([2 * 128, N // 2], bf16, name=f"ccin{T}")
                     for T in range(2)]
            cc_out = [dpool.tile([4 * 128, N // 2], bf16, name=f"ccout{T}")
                      for T in range(2)]

            # ---------------- emission state ----------------
            state = dict(pend=[], p0=None, inject=deque())

            # ---------------- QKV waves ----------------
            def w_a(f, part):  # part: 0=q 1=k 2=v
                return wa_sb[:, 384 * f + 128 * part:384 * f + 128 * (part + 1)]

            def w_pk(i, part):  # i in 0..15: 0-7 = p chunks, 8-15 = k chunks
                base = 0 if i < 8 else 3072
                ii = i % 8
                return wpk_sb[:, base + 384 * ii + 128 * part:
                              base + 384 * ii + 128 * (part + 1)]

            def emit_wave(name, fs, wof, xts, dst):
                ps = P2.tile([128, IB], f32, tag="scratch", name=name)
                for k, f in enumerate(fs):
                    nc.tensor.matmul(ps[:], wof(f), xts[f][:],
                                     start=(k == 0), stop=(k == len(fs) - 1))
                dst(ps)

            def emit_u(u):
                """x DMAs + waves KA, QA, VA, VPK + vcomb transposes for u."""
                usl = slice(512 * u, 512 * (u + 1))
                xts = {}
                for f in range(32):
                    xt = xpool.tile([128, 512], bf16, tag=f"x{f}", bufs=2,
                                    name=f"xt{f}")
                    nc.sync.dma_start(
                        xt[:], xT[(32 * u + f) * 128:(32 * u + f + 1) * 128, :])
                    xts[f] = xt

                emit_wave("ka", range(16), lambda f: w_a(f, 1), xts,
                          lambda ps: nc.vector.tensor_copy(kTa[:, usl], ps[:]))
                emit_wave("qa", range(16), lambda f: w_a(f, 0), xts,
                          lambda ps: nc.vector.tensor_copy(qTa[:, usl], ps[:]))

                comb = [ppool.tile([128, 512], bf16, tag=f"comb{c}", bufs=2,
                                   name=f"comb{c}") for c in range(2)]

                def va_dst(ps):
                    for h in range(H_LOC):
                        c, r0 = h // 2, 64 * (h % 2)
                        nc.vector.tensor_copy(comb[c][r0:r0 + 32, :],
                                              ps[32 * h:32 * h + 32, :])
                emit_wave("va", range(16), lambda f: w_a(f, 2), xts, va_dst)

                def vpk_dst(ps):
                    for h in range(H_LOC):
                        c, r0 = h // 2, 64 * (h % 2)
                        nc.vector.tensor_copy(comb[c][r0 + 32:r0 + 64, :],
                                              ps[32 * h:32 * h + 32, :])
                emit_wave("vpk", range(16, 32), lambda f: w_pk(f - 16, 2),
                          xts, vpk_dst)

                for jj in range(4):
                    j = 4 * u + jj
                    for c in range(2):
                        tp = P2.tile([128, 128], bf16, tag="scratch", name="tp")
                        nc.tensor.transpose(tp[:], comb[c][:, 128 * jj:128 * (jj + 1)],
                                            ident_bf[:])
                        nc.vector.tensor_copy(
                            vcomb[c][:, 128 * j:128 * (j + 1)], tp[:])
                return xts

            def enqueue_w2(u, xts):
                """p/k q,k projection waves as injected closures."""
                usl = slice(512 * u, 512 * (u + 1))
                specs = [("qp", range(16, 24), 0, qTp),
                         ("kp", range(16, 24), 1, kTp),
                         ("qk", range(24, 32), 0, qTk),
                         ("kk", range(24, 32), 1, kTk)]
                for name, fs, part, dstT in specs:
                    def go(name=name, fs=fs, part=part, dstT=dstT):
                        emit_wave(
                            name, fs, lambda f: w_pk(f - 16, part), xts,
                            lambda ps: nc.vector.tensor_copy(dstT[:, usl], ps[:]))
                    state["inject"].append(go)

            # ---------------- attention ----------------
            battn = [(qTa, kTa, da), (qTp, kTp, dp), (qTk, kTk, dk)]

            def mk_blk(bi, I):
                return SimpleNamespace(bi=bi, I=I,
                                       isl=slice(IB * I, IB * (I + 1)),
                                       qT=battn[bi][0], kT=battn[bi][1],
                                       d=battn[bi][2], oY=None, rP=None)

            def flush_pend():
                for blk, j, p0, p1 in state["pend"]:
                    if blk.oY is None:
                        blk.oY = [P2.tile([128, IB], f32, tag="oy", bufs=2,
                                          name=f"oY{c}") for c in range(2)]
                        blk.rP = P2.tile([128, IB], f32, tag="r", bufs=1,
                                         name="rP")
                    st, sp = (j == 0), (j == NJ - 1)
                    for half, p_sb in ((0, p0), (1, p1)):
                        prev = None
                        for hh in range(2):
                            mm = nc.tensor.matmul(
                                blk.oY[half][64 * hh:64 * (hh + 1), :],
                                vcomb[half][:, 128 * j + 64 * hh:
                                            128 * j + 64 * (hh + 1)],
                                p_sb[:, IB * hh:IB * (hh + 1)],
                                start=st, stop=sp, skip_group_check=True,
                                tile_position=(0, 64 * hh))
                            if prev is not None:
                                add_dep_helper(mm.ins, prev.ins, sync=False,
                                               reason="chain av")
                            prev = mm
                    prev = None
                    for h in range(H_LOC):
                        p_sb = p0 if h < 2 else p1
                        mm = nc.tensor.matmul(
                            blk.rP[32 * h:32 * h + 1, :],
                            ones1[:, 0:1],
                            p_sb[:, IB * (h % 2):IB * (h % 2 + 1)],
                            start=st, stop=sp, skip_group_check=True,
                            tile_position=(0, 32 * h))
                        if prev is not None:
                            add_dep_helper(mm.ins, prev.ins, sync=False,
                                           reason="chain rs")
                        prev = mm
                state["pend"] = []

            def make_epilogue(blk):
                def go():
                    for h in range(H_LOC):
                        nc.vector.tensor_copy(rrP[32 * h:32 * h + 1, :],
                                              blk.rP[32 * h:32 * h + 1, :])
                    nc.vector.reciprocal_approx_fast(rbP[:], rrP[:])
                    rbb = ppool.tile([128, IB], bf16, tag="rbb", bufs=1,
                                     name="rbb")
                    nc.vector.tensor_copy(rbb[:], rbP[:])
                    for c in range(2):
                        bc = P2.tile([128, IB], f32, tag="scratch", name="bc")
                        nc.tensor.matmul(
                            bc[:], selN_sb[:, 128 * c:128 * (c + 1)],
                            rbb[:], start=True, stop=True)
                        bcs = ppool.tile([128, IB], bf16, tag="bcs", bufs=1,
                                         name="bcs")
                        nc.vector.tensor_copy(bcs[:], bc[:])
                        if blk.bi == 0:
                            nc.vector.tensor_tensor(otc2[c][:, blk.isl],
                                                    blk.oY[c][:], bcs[:],
                                                    op=mult)
                        else:
                            tmp = ppool.tile([128, IB], f32, tag="tmp",
                                             bufs=1, name="tmp")
                            nc.vector.tensor_tensor(tmp[:], blk.oY[c][:],
                                                    bcs[:], op=mult)
                            nc.gpsimd.tensor_tensor(otc2[c][:, blk.isl],
                                                    otc2[c][:, blk.isl],
                                                    tmp[:], op=add)
                return go

            otf_t = {}

            def stage_T(T):
                hsl = slice(1024 * T, 1024 * (T + 1))
                for c in range(2):
                    ob = ppool.tile([128, 1024], bf16, tag=f"ocb{c}", bufs=1,
                                    name=f"ocb{c}")
                    nc.vector.tensor_copy(ob[:], otc2[c][:, hsl])
                    nc.sync.dma_start(cc_in[T][128 * c:128 * (c + 1), :],
                                      ob[:])
                nc.gpsimd.collective_compute(
                    "AllGather", mybir.AluOpType.bypass,
                    replica_groups=[[0, 1], [2, 3], [4, 5], [6, 7]],
                    ins=[cc_in[T].opt()], outs=[cc_out[T].opt()])
                otf = []
                for c in range(4):
                    t = ofpool.tile([128, 1024], bf16, tag=f"otf{c}", bufs=1,
                                    name=f"otf{c}")
                    nc.sync.dma_start(t[:], cc_out[T][128 * c:128 * (c + 1), :])
                    otf.append(t)
                otf_t[T] = otf
                return otf

            def make_proj(T, otf, ot, i2, tag="scratch"):
                def go():
                    i2sl = slice(512 * i2, 512 * (i2 + 1))
                    ps = P2.tile([128, 512], f32, tag=tag,
                                 bufs=(2 if tag == "oy" else 1), name="pps")
                    for ic in range(4):
                        nc.tensor.matmul(
                            ps[:],
                            wo_sb[:, 512 * ic + 128 * ot:512 * ic + 128 * (ot + 1)],
                            otf[ic][:, i2sl], start=(ic == 0), stop=(ic == 3))
                    fin = epool.tile([128, 512], bf16, tag="fin", bufs=2,
                                     name="fin")
                    nc.vector.tensor_scalar_add(fin[:], ps[:],
                                                bias_sb[:, ot:ot + 1])
                    nc.sync.dma_start(
                        out[128 * ot:128 * (ot + 1),
                            1024 * T + 512 * i2:1024 * T + 512 * (i2 + 1)],
                        fin[:])
                return go

            def emit_j(blk, j, pop=0, pre=None):
                jsl = slice(128 * j, 128 * (j + 1))
                m_t = ppool.tile([128, IB], bf16, tag="m", bufs=4, name="m_t")
                nc.gpsimd.dma_start(m_t[:], maskT[jsl, blk.isl])
                s_t = []
                prev = None
                for half in range(2):
                    s_ps = P1.tile([128, 2 * IB], f32, tag="s",
                                   name=f"sps{half}")
                    for hh in range(2):
                        h = 2 * half + hh
                        pb = 32 * h
                        mm = nc.tensor.matmul(
                            s_ps[:, IB * hh:IB * (hh + 1)],
                            blk.kT[pb:pb + blk.d, jsl],
                            blk.qT[pb:pb + blk.d, blk.isl],
                            start=True, stop=True, tile_position=(pb, 0))
                        if prev is not None:
                            add_dep_helper(mm.ins, prev.ins, sync=False,
                                           reason="chain dots")
                        prev = mm
                    s_t.append(s_ps)
                flush_pend()
                if pre is not None:
                    pre()
                for half in range(2):
                    e_sb = epool.tile([128, 2 * IB], bf16, tag="e", name="e_sb")
                    nc.scalar.activation(e_sb[:], s_t[half][:], Exp)
                    p_sb = ppool.tile([128, 2 * IB], bf16, tag="p", name="p_sb")
                    m_bc = m_t[:, None, :].broadcast_to([128, 2, IB])
                    nc.vector.tensor_tensor(
                        p_sb[:].rearrange("p (g i) -> p g i", g=2),
                        e_sb[:].rearrange("p (g i) -> p g i", g=2),
                        m_bc, op=mult)
                    if half == 0:
                        state["p0"] = p_sb
                    else:
                        state["pend"].append((blk, j, state["p0"], p_sb))
                for _ in range(pop):
                    if state["inject"]:
                        state["inject"].popleft()()

            # ---------------- main flow ----------------
            blocks = [(bi, I) for bi in range(3) for I in range(NI)]
            if KCUT == 0:
                blocks = []
                for u in range(4):
                    xts = emit_u(u)
                    enqueue_w2(u, xts)
                while state["inject"]:
                    state["inject"].popleft()()
            pending_epi = None
            pending_extra = None

            for n, (bi, I) in enumerate(blocks):
                blk = mk_blk(bi, I)
                if n == 0:
                    xts0 = emit_u(0)
                    enqueue_w2(0, xts0)
                for j in range(NJ):
                    if n == 0 and j in (4, 8, 12):
                        xts = emit_u(j // 4)
                        enqueue_w2(j // 4, xts)
                    pre = None
                    if j == 0 and (pending_epi or pending_extra):
                        pe, px = pending_epi, pending_extra

                        def pre(pe=pe, px=px):
                            if pe:
                                pe()
                            if px:
                                px()
                        pending_epi = pending_extra = None
                    pop = 1 if ((n == 0 and j >= 4) or
                                (n >= 1 and j % 2 == 1)) else 0
                    emit_j(blk, j, pop=pop, pre=pre)
                if KCUT >= 2:
                    pending_epi = make_epilogue(blk)
                if KCUT >= 3 and n == 9:  # (2,1): token half T0 now final
                    def extra():
                        stage_T(0)
                    pending_extra = extra
                if KCUT >= 3 and n == 10:
                    for ot in range(4):
                        for i2 in range(2):
                            state["inject"].append(
                                make_proj(0, otf_t[0], ot, i2))

            if blocks:
                flush_pend()
                if pending_epi:
                    pending_epi()
                while state["inject"]:
                    state["inject"].popleft()()
            if KCUT >= 3:
                otf1 = stage_T(1)
                for ot in range(4):
                    for i2 in range(2):
                        make_proj(1, otf1, ot, i2, tag="oy")()

    nc.compile()
    return nc


def _prep_core(c, x, W_a, W_p, W_k, W_out, b_out, mask):
    b = c // 2
    h0 = H_LOC * (c % 2)

    xTb = np.ascontiguousarray(x[b].T).astype(BF16)      # [4096, 2048]
    # tile into [u, f, 128, 512] -> [16384, 512]
    xt = xTb.reshape(32, 128, 4, 512).transpose(2, 0, 1, 3)
    xt = np.ascontiguousarray(xt.reshape(4 * 32 * 128, 512))

    maskTb = np.ascontiguousarray(mask[b, 0].T).astype(BF16)

    qa = W_a[da * h0: da * (h0 + H_LOC), :] * (DA ** -0.5)
    ka = W_a[DA_H + da * h0: DA_H + da * (h0 + H_LOC), :]
    va = W_a[2 * DA_H + da * h0: 2 * DA_H + da * (h0 + H_LOC), :]
    waT = np.concatenate([qa.T, ka.T, va.T], axis=1).astype(np.float32)
    # [2048, 384] -> stacked [128, 16*384]
    wa_all = waT.reshape(16, 128, 384).transpose(1, 0, 2).reshape(128, 6144)
    wa_all = np.ascontiguousarray(wa_all).astype(BF16)

    def pk_branch(W, D, D_H, d, vcol_ofs):
        qpad = np.zeros((D, 128), np.float32)
        kpad = np.zeros((D, 128), np.float32)
        vpad = np.zeros((D, 128), np.float32)
        for h in range(H_LOC):
            qpad[:, 32 * h:32 * h + d] = \
                W[d * (h0 + h): d * (h0 + h + 1), :].T * (D ** -0.5)
            kpad[:, 32 * h:32 * h + d] = \
                W[D_H + d * (h0 + h): D_H + d * (h0 + h + 1), :].T
            vpad[:, 32 * h + vcol_ofs:32 * h + vcol_ofs + d] = \
                W[2 * D_H + d * (h0 + h): 2 * D_H + d * (h0 + h + 1), :].T
        wT = np.concatenate([qpad, kpad, vpad], axis=1)   # [D, 384]
        return wT.reshape(8, 128, 384).transpose(1, 0, 2).reshape(128, 3072)

    wp_all = pk_branch(W_p, DP, DP_H, dp, 0)
    wk_all = pk_branch(W_k, DK, DK_H, dk, 16)
    wpk_all = np.ascontiguousarray(
        np.concatenate([wp_all, wk_all], axis=1)).astype(BF16)

    woutT = np.ascontiguousarray((W_out / 3.0).T).astype(np.float32)
    # [512, 512] -> [128, 4*512]
    wo_all = woutT.reshape(4, 128, 512).transpose(1, 0, 2).reshape(128, 2048)
    wo_all = np.ascontiguousarray(wo_all).astype(BF16)

    bout = np.ascontiguousarray(b_out.reshape(DOUT, 1)).astype(np.float32)

    # selc[:, 128c:128(c+1)] broadcasts r rows {32*2c, 32*(2c+1)} to the
    # 64-col blocks of bc: bc_c[m, q] = r[32*(2c + m//64)][q]
    selc = np.zeros((128, 256), np.float32)
    for c in range(2):
        for hh in range(2):
            selc[32 * (2 * c + hh), 128 * c + 64 * hh:
                 128 * c + 64 * hh + 64] = 1.0
    selc = np.ascontiguousarray(selc).astype(BF16)

    return {
        "selc": selc,
        "xT": xt,
        "maskT": maskTb,
        "wa": wa_all,
        "wpk": wpk_all,
        "wo": wo_all,
        "bout": bout,
    }


def kernel(x, W_a, W_p, W_k, W_out, b_out, mask):
    from concourse.bass_utils import run_bass_kernel_spmd

    x = np.asarray(x, np.float32)
    W_a = np.asarray(W_a, np.float32)
    W_p = np.asarray(W_p, np.float32)
    W_k = np.asarray(W_k, np.float32)
    W_out = np.asarray(W_out, np.float32)
    b_out = np.asarray(b_out, np.float32)
    mask = np.asarray(mask)

    if "nc" not in _CACHE:
        _CACHE["nc"] = _build()
    nc = _CACHE["nc"]

    in_maps = [_prep_core(c, x, W_a, W_p, W_k, W_out, b_out, mask)
               for c in range(NCORES)]
    res = run_bass_kernel_spmd(nc, in_maps, core_ids=list(range(NCORES)))

    outs = []
    for b in range(B):
        outs.append(np.asarray(res.results[2 * b]["out"]).astype(np.float32).T)
    return np.stack(outs, axis=0)
